# revision 11
# baseline (speedup 1.0000x reference)
"""AttentionLSTM (2-layer enc/dec LSTM + dot-product attention) on 8 trn2 NeuronCores.

Sharding: data-parallel over batch (B=64 -> 8 cores x 8). Per core:
  - On-chip layout is feature-major: hidden state h kept as (h, b) tiles so the
    recurrent matmuls (gates-stationary, bf16 weights) need no transposes.
  - Input projections gx = Wih @ x^T + b precomputed in bulk -> DRAM, streamed
    back per step.
  - Encoder 512 steps -> encT; decoder 512 steps -> decT (wavefront across the
    two layers); then attention (scores/softmax/attn_v) per batch element.
Host does all weight/input layout preprocessing and output reassembly.
"""
import sys

import numpy as np

for _p in ("/opt/trn_rl_repo", "/root/.axon_site/_ro/trn_rl_repo"):
    if _p not in sys.path:
        sys.path.append(_p)

import ml_dtypes  # noqa: E402
import concourse.bass as bass  # noqa: E402
import concourse.bacc as bacc  # noqa: E402
import concourse.mybir as mybir  # noqa: E402
from concourse import tile  # noqa: E402
from concourse.bass_utils import run_bass_kernel_spmd  # noqa: E402

F32 = mybir.dt.float32
BF16 = mybir.dt.bfloat16
AF = mybir.ActivationFunctionType
ALU = mybir.AluOpType
AX = mybir.AxisListType

NCORES = 8
S, T, B = 512, 512, 64
BL = B // NCORES          # 8 batch per core
H = 256
IN = 256
G = 4 * H                 # 1024 gates
KH = H // 128             # 2 k-tiles for hidden contraction
KHG = (H + H) // 128      # 4 k-tiles for layer-1 contraction [h0; h1]
M8 = G // 128             # 8 gate m-tiles
CH = 8                    # recurrence steps per gx DMA chunk

# torch gate order (i, f, g, o) -> on-chip order (i, f, o, g) so one sigmoid
# covers cols [0, 6*BL) and one tanh covers [6*BL, 8*BL)
GATE_ORDER = np.concatenate(
    [np.arange(0, 2 * H), np.arange(3 * H, 4 * H), np.arange(2 * H, 3 * H)]
)


def build_nc(ns=S, nt=T):
    nc = bacc.Bacc("TRN2", target_bir_lowering=False, debug=False,
                   num_devices=NCORES)

    def inp(name, shape, dt):
        return nc.dram_tensor(name, list(shape), dt, kind="ExternalInput")

    xte = inp("xte", (128, KH * ns * BL), BF16)      # col = k*(ns*BL) + t*BL + b
    xtd = inp("xtd", (128, KH * nt * BL), BF16)
    w0e = inp("w0e", (128, KH * M8 * 128), BF16)     # lhsT tiles, col j = m*KH + k
    w1e = inp("w1e", (128, KHG * M8 * 128), BF16)    # j = m*KHG + k
    w0d = inp("w0d", (128, KH * M8 * 128), BF16)
    w1d = inp("w1d", (128, KHG * M8 * 128), BF16)
    wie = inp("wie", (128, KH * M8 * 128), BF16)     # Wih0^T tiles for prologue
    wid = inp("wid", (128, KH * M8 * 128), BF16)
    b0e = inp("b0e", (128, M8), F32)
    b0d = inp("b0d", (128, M8), F32)
    b1e = inp("b1e", (128, M8 * BL), BF16)
    b1d = inp("b1d", (128, M8 * BL), BF16)
    ident = inp("ident", (128, 128), F32)
    identb = inp("identb", (128, 128), BF16)

    dect = nc.dram_tensor("dect", [128, nt * 2 * BL], BF16, kind="ExternalOutput")
    attnv = nc.dram_tensor("attnv", [BL, nt, H], F32, kind="ExternalOutput")
    attnw = nc.dram_tensor("attnw", [BL, nt, ns], F32, kind="ExternalOutput")

    with tile.TileContext(nc) as tc:
        with (
            tc.tile_pool(name="const", bufs=1) as constp,
            tc.tile_pool(name="store", bufs=1) as storep,
            tc.tile_pool(name="dram", bufs=1, space="DRAM") as dramp,
            tc.tile_pool(name="state", bufs=2) as statep,
            tc.tile_pool(name="gxp", bufs=4) as gxp,
            tc.tile_pool(name="work", bufs=3) as workp,
        ):
            def load_const(dram_t, shape, dt):
                t = constp.tile(shape, dt, name=dram_t.name + "_sb")
                nc.sync.dma_start(t[:], dram_t[:])
                return t

            w0e_sb = load_const(w0e, [128, KH * M8 * 128], BF16)
            w1e_sb = load_const(w1e, [128, KHG * M8 * 128], BF16)
            w0d_sb = load_const(w0d, [128, KH * M8 * 128], BF16)
            w1d_sb = load_const(w1d, [128, KHG * M8 * 128], BF16)
            wie_sb = load_const(wie, [128, KH * M8 * 128], BF16)
            wid_sb = load_const(wid, [128, KH * M8 * 128], BF16)
            b0e_sb = load_const(b0e, [128, M8], F32)
            b0d_sb = load_const(b0d, [128, M8], F32)
            b1e_sb = load_const(b1e, [128, M8 * BL], BF16)
            b1d_sb = load_const(b1d, [128, M8 * BL], BF16)
            ident_sb = load_const(ident, [128, 128], F32)
            identb_sb = load_const(identb, [128, 128], BF16)

            encT = storep.tile([128, ns * 2 * BL], BF16)  # col = t*16 + k*8 + b
            decT = storep.tile([128, nt * 2 * BL], BF16)

            gxe_d = dramp.tile([128, ns, M8 * BL], BF16)
            gxd_d = dramp.tile([128, nt, M8 * BL], BF16)

            # ---------------- prologue: gx = Wih0 @ x^T + b0 -> DRAM ----------
            with (
                tc.tile_pool(name="prolog", bufs=2) as prop,
                tc.tile_pool(name="propsum", bufs=4, space="PSUM") as props,
            ):
                for xt_dram, wi_sb, b0_sb, gx_d, n in (
                    (xte, wie_sb, b0e_sb, gxe_d, ns),
                    (xtd, wid_sb, b0d_sb, gxd_d, nt),
                ):
                    xt_sb = prop.tile([128, KH * n * BL], BF16, tag="xt")
                    nc.sync.dma_start(xt_sb[:], xt_dram[:])
                    nchunks = (n * BL) // 512    # 64 steps per chunk
                    tpc = 512 // BL              # steps per chunk
                    for c in range(nchunks):
                        sg = prop.tile([128, tpc, M8 * BL], BF16, tag="sg")
                        for m in range(M8):
                            ps = props.tile([128, 512], F32, tag="pp")
                            for k in range(KH):
                                j = m * KH + k
                                nc.tensor.matmul(
                                    ps[:],
                                    wi_sb[:, j * 128:(j + 1) * 128],
                                    xt_sb[:, k * n * BL + c * 512:
                                          k * n * BL + (c + 1) * 512],
                                    start=(k == 0), stop=(k == KH - 1),
                                )
                            ps3 = ps[:].rearrange("p (t b) -> p t b", b=BL)
                            dst = sg[:, :, m * BL:(m + 1) * BL]
                            if m % 2 == 0:
                                nc.scalar.activation(
                                    dst, ps3, AF.Identity, bias=b0_sb[:, m:m + 1]
                                )
                            else:
                                nc.vector.tensor_scalar_add(
                                    dst, ps3, b0_sb[:, m:m + 1]
                                )
                        nc.sync.dma_start(gx_d[:, c * tpc:(c + 1) * tpc, :], sg[:])

            # ---------------- recurrences ------------------------------------
            def zeros_state():
                h0 = statep.tile([128, KH * BL], BF16, tag="h0")
                c0 = statep.tile([128, KH * BL], F32, tag="ac")
                h1 = statep.tile([128, KH * BL], BF16, tag="h1z")
                c1 = statep.tile([128, KH * BL], F32, tag="bc")
                for t_ in (h0, c0, h1, c1):
                    nc.gpsimd.memset(t_[:], 0.0)
                return {"h0T": h0, "c0": c0, "h1": h1[:], "c1": c1,
                        "h0_hist": {-1: h0}}

            def gates(z, c_prev, tag, h_out):
                """z (128, 8*BL) PSUM pre-activations [i f o g] -> writes
                h into h_out AP, returns c_new."""
                s = workp.tile([128, 6 * BL], F32, tag=tag + "s")
                nc.scalar.activation(s[:], z[:, 0:6 * BL], AF.Sigmoid)
                g = workp.tile([128, 2 * BL], F32, tag=tag + "g")
                nc.scalar.activation(g[:], z[:, 6 * BL:8 * BL], AF.Tanh)
                t1 = workp.tile([128, 2 * BL], F32, tag=tag + "t1")
                nc.vector.tensor_mul(t1[:], s[:, 0:2 * BL], g[:])
                u = workp.tile([128, 2 * BL], F32, tag=tag + "u")
                nc.vector.tensor_mul(u[:], s[:, 2 * BL:4 * BL], c_prev[:])
                c_new = statep.tile([128, 2 * BL], F32, tag=tag + "c")
                nc.vector.tensor_add(c_new[:], u[:], t1[:])
                tch = workp.tile([128, 2 * BL], F32, tag=tag + "tc")
                nc.scalar.activation(tch[:], c_new[:], AF.Tanh)
                nc.vector.tensor_mul(h_out, s[:, 4 * BL:6 * BL], tch[:])
                return c_new

            def recurrence(w0_sb, w1_sb, b1_sb, gx_d, n, outT, st, psA, psB):
                """Wavefront: iteration t emits L0 step t and L1 step t-1 so
                the PE always has independent work while L0's elementwise
                chain completes.  h1 state lives directly in outT (bf16)."""
                h0T, c0, c1 = st["h0T"], st["c0"], st["c1"]
                h1_prev = st["h1"]      # AP (128, 16) bf16 (prev LSTM's final)
                nch = (n + CH - 1) // CH
                gx_tiles = {}

                def issue_gx(c):
                    if c < nch:
                        gt = gxp.tile([128, CH, M8 * BL], BF16, tag="gx")
                        nc.sync.dma_start(gt[:], gx_d[:, c * CH:(c + 1) * CH, :])
                        gx_tiles[c] = gt

                for c in range(min(3, nch)):
                    issue_gx(c)

                z0_t = {}
                for t in range(n + 1):
                    z1 = None
                    if t < n:
                        if t % CH == 0:
                            issue_gx(t // CH + 3)
                        gxt = gx_tiles[t // CH][:, t % CH, :]
                        # L0 matmuls: gx preload via identity, then weights
                        z0 = psA.tile([128, M8 * BL], F32, tag="z0")
                        nc.tensor.matmul(z0[:], identb_sb[:], gxt,
                                         start=True, stop=False)
                        for m in range(M8):
                            for k in range(KH):
                                j = m * KH + k
                                nc.tensor.matmul(
                                    z0[:, m * BL:(m + 1) * BL],
                                    w0_sb[:, j * 128:(j + 1) * 128],
                                    h0T[:, k * BL:(k + 1) * BL],
                                    start=False,
                                    stop=(m == M8 - 1 and k == KH - 1),
                                )
                        z0_t[t] = z0
                    if t >= 1:
                        # L1 matmuls for step t-1; h1-reading MMs (k>=KH)
                        # emitted last to relax the h1 deadline.
                        tp = t - 1
                        h0_in = st["h0_hist"][tp]
                        h1_in = (outT[:, (tp - 1) * 2 * BL:tp * 2 * BL]
                                 if tp >= 1 else h1_prev)
                        z1 = psB.tile([128, M8 * BL], F32, tag="z1")
                        nc.tensor.matmul(z1[:], identb_sb[:], b1_sb[:],
                                         start=True, stop=False)
                        for k in range(KHG):
                            rhs_all = (h0_in[:, k * BL:(k + 1) * BL] if k < KH
                                       else h1_in[:, (k - KH) * BL:
                                                  (k - KH + 1) * BL])
                            for m in range(M8):
                                j = m * KHG + k
                                nc.tensor.matmul(
                                    z1[:, m * BL:(m + 1) * BL],
                                    w1_sb[:, j * 128:(j + 1) * 128],
                                    rhs_all,
                                    start=False,
                                    stop=(m == M8 - 1 and k == KHG - 1),
                                )
                    if t < n:
                        h0n = statep.tile([128, 2 * BL], BF16, tag="h0")
                        c0 = gates(z0_t.pop(t), c0, "a", h0n[:])
                        st["h0_hist"][t] = h0n
                        if t - 2 in st["h0_hist"]:
                            del st["h0_hist"][t - 2]
                        h0T = h0n
                    if z1 is not None:
                        tp = t - 1
                        c1 = gates(z1, c1, "b",
                                   outT[:, tp * 2 * BL:(tp + 1) * 2 * BL])
                st["h0T"], st["c0"], st["c1"] = h0T, c0, c1
                st["h1"] = outT[:, (n - 1) * 2 * BL:n * 2 * BL]
                return st

            with (
                tc.tile_pool(name="psA", bufs=2, space="PSUM") as psA,
                tc.tile_pool(name="psB", bufs=2, space="PSUM") as psB,
            ):
                st = zeros_state()
                st = recurrence(w0e_sb, w1e_sb, b1e_sb, gxe_d, ns, encT, st,
                                psA, psB)
                st = recurrence(w0d_sb, w1d_sb, b1d_sb, gxd_d, nt, decT, st,
                                psA, psB)

            # ---------------- attention -------------------------------------
            nsc = ns // 128
            ntc = nt // 128
            with (
                tc.tile_pool(name="attn", bufs=2) as attp,
                tc.tile_pool(name="attnc", bufs=1) as attc,
                tc.tile_pool(name="attps", bufs=2, space="PSUM") as attps,
            ):
                enc_nat = attc.tile([128, BL, nsc, H], F32)
                encT4 = encT[:].rearrange("p (s k b) -> p s k b", k=KH, b=BL)
                decT4 = decT[:].rearrange("p (s k b) -> p s k b", k=KH, b=BL)
                for b in range(BL):
                    for k in range(KH):
                        for sc in range(nsc):
                            pt = attps.tile([128, 128], F32, tag="pt")
                            nc.tensor.transpose(
                                pt[:], encT4[:, sc * 128:(sc + 1) * 128, k, b],
                                ident_sb[:],
                            )
                            nc.scalar.copy(
                                enc_nat[:, b, sc, k * 128:(k + 1) * 128], pt[:]
                            )
                    for tcn in range(ntc):
                        ps_s = attps.tile([128, ns], F32, tag="ps_s")
                        for k in range(KH):
                            nc.tensor.matmul(
                                ps_s[:],
                                decT4[:, tcn * 128:(tcn + 1) * 128, k, b],
                                encT4[:, :, k, b],
                                start=(k == 0), stop=(k == KH - 1),
                            )
                        nmx = attp.tile([128, 1], F32, tag="nmx")
                        nc.vector.tensor_reduce(
                            nmx[:], ps_s[:], axis=AX.X, op=ALU.max, negate=True
                        )
                        wexp = attp.tile([128, ns], F32, tag="wexp")
                        den = attp.tile([128, 1], F32, tag="den")
                        nc.scalar.activation(
                            wexp[:], ps_s[:], AF.Exp, bias=nmx[:],
                            accum_out=den[:],
                        )
                        rden = attp.tile([128, 1], F32, tag="rden")
                        nc.vector.reciprocal(rden[:], den[:])
                        wn = attp.tile([128, ns], F32, tag="wn")
                        nc.vector.tensor_scalar_mul(wn[:], wexp[:], rden[:])
                        nc.sync.dma_start(
                            attnw[b, tcn * 128:(tcn + 1) * 128, :], wn[:]
                        )
                        wT = attp.tile([128, nsc * 128], F32, tag="wT")
                        for j in range(nsc):
                            ptw = attps.tile([128, 128], F32, tag="pt")
                            nc.tensor.transpose(
                                ptw[:], wn[:, j * 128:(j + 1) * 128], ident_sb[:]
                            )
                            nc.scalar.copy(wT[:, j * 128:(j + 1) * 128], ptw[:])
                        ps_v = attps.tile([128, H], F32, tag="ps_v")
                        for j in range(nsc):
                            nc.tensor.matmul(
                                ps_v[:],
                                wT[:, j * 128:(j + 1) * 128],
                                enc_nat[:, b, j, :],
                                start=(j == 0), stop=(j == nsc - 1),
                            )
                        vsb = attp.tile([128, H], F32, tag="vsb")
                        nc.scalar.copy(vsb[:], ps_v[:])
                        nc.sync.dma_start(
                            attnv[b, tcn * 128:(tcn + 1) * 128, :], vsb[:]
                        )
                nc.sync.dma_start(dect[:], decT[:])
    nc.compile()
    return nc


# ---------------------- host-side layout helpers ----------------------------

def _prep_xt(x):
    """(n, BL, 256) f32 -> (128, 2*n*BL) bf16, col = k*(n*BL) + t*BL + b."""
    n = x.shape[0]
    a = np.ascontiguousarray(x.transpose(2, 0, 1)).reshape(KH, 128, n * BL)
    return np.concatenate([a[0], a[1]], axis=1).astype(ml_dtypes.bfloat16)


def _prep_lhsT(Wp):
    """Permuted weight (1024, Kdim) -> (128, KT*8*128) bf16 lhsT tiles,
    col block j = m*KT + k."""
    Kd = Wp.shape[1]
    KT = Kd // 128
    t4 = np.ascontiguousarray(Wp.T).reshape(KT, 128, M8, 128)
    return np.ascontiguousarray(
        t4.transpose(1, 2, 0, 3)
    ).reshape(128, M8 * KT * 128).astype(ml_dtypes.bfloat16)


def _prep_shared(inputs, ns, nt):
    f = lambda k: np.asarray(inputs[k], np.float32)
    sh = {}
    for tag, wih, whh, bih, bhh in (
        ("e", f("enc_Wih"), f("enc_Whh"), f("enc_bih"), f("enc_bhh")),
        ("d", f("dec_Wih"), f("dec_Whh"), f("dec_bih"), f("dec_bhh")),
    ):
        sh["w0" + tag] = _prep_lhsT(whh[0][GATE_ORDER])
        sh["w1" + tag] = _prep_lhsT(
            np.concatenate([wih[1], whh[1]], axis=1)[GATE_ORDER]
        )
        sh["wi" + tag] = _prep_lhsT(wih[0][GATE_ORDER])
        b0 = (bih[0] + bhh[0])[GATE_ORDER]
        sh["b0" + tag] = np.ascontiguousarray(b0.reshape(M8, 128).T)
        b1 = (bih[1] + bhh[1])[GATE_ORDER]
        b1r = b1.reshape(M8, 128).T          # (128, 8)
        sh["b1" + tag] = np.ascontiguousarray(
            np.repeat(b1r[:, :, None], BL, axis=2).reshape(128, M8 * BL)
        )
    sh["ident"] = np.eye(128, dtype=np.float32)
    return sh


_BUILT = {}


def _get_nc(ns, nt):
    key = (ns, nt)
    if key not in _BUILT:
        _BUILT[key] = build_nc(ns, nt)
    return _BUILT[key]


def run(inputs, ns=S, nt=T):
    """Run the kernel; returns (responses, attn_w) full-shape."""
    nc = _get_nc(ns, nt)
    enc_in = np.asarray(inputs["enc_input"], np.float32)[:ns]
    dec_in = np.asarray(inputs["dec_input"], np.float32)[:nt]
    nb = enc_in.shape[1]
    ncores = nb // BL
    shared = _prep_shared(inputs, ns, nt)
    in_maps = []
    for c in range(ncores):
        m = dict(shared)
        sl = slice(c * BL, (c + 1) * BL)
        m["xte"] = _prep_xt(enc_in[:, sl, :])
        m["xtd"] = _prep_xt(dec_in[:, sl, :])
        in_maps.append(m)
    res = run_bass_kernel_spmd(nc, in_maps, list(range(ncores)))
    resp = np.empty((nt, nb, 2 * H), np.float32)
    attw = np.empty((nt, nb, ns), np.float32)
    for c in range(ncores):
        r = res.results[c]
        sl = slice(c * BL, (c + 1) * BL)
        dect = r["dect"].reshape(128, nt, KH, BL)
        resp[:, sl, 0:H] = np.ascontiguousarray(
            dect.transpose(1, 3, 2, 0)
        ).reshape(nt, BL, H)
        resp[:, sl, H:2 * H] = r["attnv"].transpose(1, 0, 2)
        attw[:, sl, :] = r["attnw"].transpose(1, 0, 2)
    return resp, attw


def kernel(**inputs):
    return run(inputs, S, T)


# revision 14
# speedup vs baseline: 1869.5858x; 1869.5858x over previous
"""AttentionLSTM (2-layer enc/dec LSTM + dot-product attention) on 8 trn2 NeuronCores.

Sharding: data-parallel over batch (B=64 -> 8 cores x 8). Per core:
  - On-chip layout is feature-major: hidden state h kept as (h, b) tiles so the
    recurrent matmuls (gates-stationary, bf16 weights) need no transposes.
  - Input projections gx = Wih @ x^T + b precomputed in bulk -> DRAM, streamed
    back per step.
  - Encoder 512 steps -> encT; decoder 512 steps -> decT (wavefront across the
    two layers); then attention (scores/softmax/attn_v) per batch element.
Host does all weight/input layout preprocessing and output reassembly.
"""
import sys

import numpy as np

for _p in ("/opt/trn_rl_repo", "/root/.axon_site/_ro/trn_rl_repo"):
    if _p not in sys.path:
        sys.path.append(_p)

import ml_dtypes  # noqa: E402
import concourse.bass as bass  # noqa: E402
import concourse.bacc as bacc  # noqa: E402
import concourse.mybir as mybir  # noqa: E402
from concourse import tile  # noqa: E402
from concourse.bass_utils import run_bass_kernel_spmd  # noqa: E402

F32 = mybir.dt.float32
BF16 = mybir.dt.bfloat16
AF = mybir.ActivationFunctionType
ALU = mybir.AluOpType
AX = mybir.AxisListType

NCORES = 8
S, T, B = 512, 512, 64
BL = B // NCORES          # 8 batch per core
H = 256
IN = 256
G = 4 * H                 # 1024 gates
KH = H // 128             # 2 k-tiles for hidden contraction
KHG = (H + H) // 128      # 4 k-tiles for layer-1 contraction [h0; h1]
M8 = G // 128             # 8 gate m-tiles
CH = 8                    # recurrence steps per gx DMA chunk

# torch gate order (i, f, g, o) -> on-chip order (i, f, o, g) so one sigmoid
# covers cols [0, 6*BL) and one tanh covers [6*BL, 8*BL)
GATE_ORDER = np.concatenate(
    [np.arange(0, 2 * H), np.arange(3 * H, 4 * H), np.arange(2 * H, 3 * H)]
)


def build_nc(ns=S, nt=T):
    nc = bacc.Bacc("TRN2", target_bir_lowering=False, debug=False,
                   num_devices=NCORES)

    def inp(name, shape, dt):
        return nc.dram_tensor(name, list(shape), dt, kind="ExternalInput")

    xte = inp("xte", (128, KH * ns * BL), BF16)      # col = k*(ns*BL) + t*BL + b
    xtd = inp("xtd", (128, KH * nt * BL), BF16)
    w0e = inp("w0e", (128, KH * M8 * 128), BF16)     # lhsT tiles, col j = m*KH + k
    w1e = inp("w1e", (128, KHG * M8 * 128), BF16)    # j = m*KHG + k
    w0d = inp("w0d", (128, KH * M8 * 128), BF16)
    w1d = inp("w1d", (128, KHG * M8 * 128), BF16)
    wie = inp("wie", (128, KH * M8 * 128), BF16)     # Wih0^T tiles for prologue
    wid = inp("wid", (128, KH * M8 * 128), BF16)
    b0e = inp("b0e", (128, M8), F32)
    b0d = inp("b0d", (128, M8), F32)
    b1e = inp("b1e", (128, M8 * BL), BF16)
    b1d = inp("b1d", (128, M8 * BL), BF16)
    ident = inp("ident", (128, 128), F32)
    identb = inp("identb", (128, 128), BF16)

    dect = nc.dram_tensor("dect", [128, nt * 2 * BL], BF16, kind="ExternalOutput")
    attnv = nc.dram_tensor("attnv", [BL, nt, H], F32, kind="ExternalOutput")
    attnw = nc.dram_tensor("attnw", [BL, nt, ns], F32, kind="ExternalOutput")

    with tile.TileContext(nc) as tc:
        with (
            tc.tile_pool(name="const", bufs=1) as constp,
            tc.tile_pool(name="store", bufs=1) as storep,
            tc.tile_pool(name="dram", bufs=1, space="DRAM") as dramp,
            tc.tile_pool(name="state", bufs=2) as statep,
            tc.tile_pool(name="gxp", bufs=4) as gxp,
            tc.tile_pool(name="work", bufs=3) as workp,
        ):
            def load_const(dram_t, shape, dt):
                t = constp.tile(shape, dt, name=dram_t.name + "_sb")
                nc.sync.dma_start(t[:], dram_t[:])
                return t

            w0e_sb = load_const(w0e, [128, KH * M8 * 128], BF16)
            w1e_sb = load_const(w1e, [128, KHG * M8 * 128], BF16)
            w0d_sb = load_const(w0d, [128, KH * M8 * 128], BF16)
            w1d_sb = load_const(w1d, [128, KHG * M8 * 128], BF16)
            wie_sb = load_const(wie, [128, KH * M8 * 128], BF16)
            wid_sb = load_const(wid, [128, KH * M8 * 128], BF16)
            b0e_sb = load_const(b0e, [128, M8], F32)
            b0d_sb = load_const(b0d, [128, M8], F32)
            b1e_sb = load_const(b1e, [128, M8 * BL], BF16)
            b1d_sb = load_const(b1d, [128, M8 * BL], BF16)
            ident_sb = load_const(ident, [128, 128], F32)
            identb_sb = load_const(identb, [128, 128], BF16)

            encT = storep.tile([128, ns * 2 * BL], BF16)  # col = t*16 + k*8 + b
            decT = storep.tile([128, nt * 2 * BL], BF16)

            gxe_d = dramp.tile([128, ns, M8 * BL], BF16)
            gxd_d = dramp.tile([128, nt, M8 * BL], BF16)

            # ---------------- prologue: gx = Wih0 @ x^T + b0 -> DRAM ----------
            with (
                tc.tile_pool(name="prolog", bufs=2) as prop,
                tc.tile_pool(name="propsum", bufs=4, space="PSUM") as props,
            ):
                for xt_dram, wi_sb, b0_sb, gx_d, n in (
                    (xte, wie_sb, b0e_sb, gxe_d, ns),
                    (xtd, wid_sb, b0d_sb, gxd_d, nt),
                ):
                    xt_sb = prop.tile([128, KH * n * BL], BF16, tag="xt")
                    nc.sync.dma_start(xt_sb[:], xt_dram[:])
                    nchunks = (n * BL) // 512    # 64 steps per chunk
                    tpc = 512 // BL              # steps per chunk
                    for c in range(nchunks):
                        sg = prop.tile([128, tpc, M8 * BL], BF16, tag="sg")
                        for m in range(M8):
                            ps = props.tile([128, 512], F32, tag="pp")
                            for k in range(KH):
                                j = m * KH + k
                                nc.tensor.matmul(
                                    ps[:],
                                    wi_sb[:, j * 128:(j + 1) * 128],
                                    xt_sb[:, k * n * BL + c * 512:
                                          k * n * BL + (c + 1) * 512],
                                    start=(k == 0), stop=(k == KH - 1),
                                )
                            ps3 = ps[:].rearrange("p (t b) -> p t b", b=BL)
                            dst = sg[:, :, m * BL:(m + 1) * BL]
                            if m % 2 == 0:
                                nc.scalar.activation(
                                    dst, ps3, AF.Identity, bias=b0_sb[:, m:m + 1]
                                )
                            else:
                                nc.vector.tensor_scalar_add(
                                    dst, ps3, b0_sb[:, m:m + 1]
                                )
                        nc.sync.dma_start(gx_d[:, c * tpc:(c + 1) * tpc, :], sg[:])

            # ---------------- recurrences ------------------------------------
            def zeros_state():
                h0 = statep.tile([128, KH * BL], BF16, tag="h0")
                c0 = statep.tile([128, KH * BL], F32, tag="ac")
                h1 = statep.tile([128, KH * BL], BF16, tag="h1z")
                c1 = statep.tile([128, KH * BL], F32, tag="bc")
                for t_ in (h0, c0, h1, c1):
                    nc.gpsimd.memset(t_[:], 0.0)
                return {"h0T": h0, "c0": c0, "h1": h1[:], "c1": c1,
                        "h0_hist": {-1: h0}}

            def gates(z, c_prev, tag, h_out):
                """z (128, 8*BL) PSUM pre-activations [i f o g] -> writes
                h into h_out AP, returns c_new."""
                s = workp.tile([128, 6 * BL], F32, tag=tag + "s")
                nc.scalar.activation(s[:], z[:, 0:6 * BL], AF.Sigmoid)
                g = workp.tile([128, 2 * BL], F32, tag=tag + "g")
                nc.scalar.activation(g[:], z[:, 6 * BL:8 * BL], AF.Tanh)
                t1 = workp.tile([128, 2 * BL], F32, tag=tag + "t1")
                nc.vector.tensor_mul(t1[:], s[:, 0:2 * BL], g[:])
                u = workp.tile([128, 2 * BL], F32, tag=tag + "u")
                nc.vector.tensor_mul(u[:], s[:, 2 * BL:4 * BL], c_prev[:])
                c_new = statep.tile([128, 2 * BL], F32, tag=tag + "c")
                nc.vector.tensor_add(c_new[:], u[:], t1[:])
                tch = workp.tile([128, 2 * BL], F32, tag=tag + "tc")
                nc.scalar.activation(tch[:], c_new[:], AF.Tanh)
                nc.vector.tensor_mul(h_out, s[:, 4 * BL:6 * BL], tch[:])
                return c_new

            def recurrence(w0_sb, w1_sb, b1_sb, gx_d, n, outT, st, psA, psB):
                """Wavefront: iteration t emits L0 step t and L1 step t-1 so
                the PE always has independent work while L0's elementwise
                chain completes.  h1 state lives directly in outT (bf16)."""
                h0T, c0, c1 = st["h0T"], st["c0"], st["c1"]
                h1_prev = st["h1"]      # AP (128, 16) bf16 (prev LSTM's final)
                nch = (n + CH - 1) // CH
                gx_tiles = {}

                def issue_gx(c):
                    if c < nch:
                        gt = gxp.tile([128, CH, M8 * BL], BF16, tag="gx")
                        nc.sync.dma_start(gt[:], gx_d[:, c * CH:(c + 1) * CH, :])
                        gx_tiles[c] = gt

                for c in range(min(3, nch)):
                    issue_gx(c)

                z0_t = {}
                for t in range(n + 1):
                    z1 = None
                    if t < n:
                        if t % CH == 0:
                            issue_gx(t // CH + 3)
                        gxt = gx_tiles[t // CH][:, t % CH, :]
                        # L0 matmuls: gx preload via identity, then weights
                        z0 = psA.tile([128, M8 * BL], F32, tag="z0")
                        nc.tensor.matmul(z0[:], identb_sb[:], gxt,
                                         start=True, stop=False)
                        for m in range(M8):
                            for k in range(KH):
                                j = m * KH + k
                                nc.tensor.matmul(
                                    z0[:, m * BL:(m + 1) * BL],
                                    w0_sb[:, j * 128:(j + 1) * 128],
                                    h0T[:, k * BL:(k + 1) * BL],
                                    start=False,
                                    stop=(m == M8 - 1 and k == KH - 1),
                                )
                        z0_t[t] = z0
                    if t >= 1:
                        # L1 matmuls for step t-1; h1-reading MMs (k>=KH)
                        # emitted last to relax the h1 deadline.
                        tp = t - 1
                        h0_in = st["h0_hist"][tp]
                        h1_in = (outT[:, (tp - 1) * 2 * BL:tp * 2 * BL]
                                 if tp >= 1 else h1_prev)
                        z1 = psB.tile([128, M8 * BL], F32, tag="z1")
                        nc.tensor.matmul(z1[:], identb_sb[:], b1_sb[:],
                                         start=True, stop=False)
                        for k in range(KHG):
                            rhs_all = (h0_in[:, k * BL:(k + 1) * BL] if k < KH
                                       else h1_in[:, (k - KH) * BL:
                                                  (k - KH + 1) * BL])
                            for m in range(M8):
                                j = m * KHG + k
                                nc.tensor.matmul(
                                    z1[:, m * BL:(m + 1) * BL],
                                    w1_sb[:, j * 128:(j + 1) * 128],
                                    rhs_all,
                                    start=False,
                                    stop=(m == M8 - 1 and k == KHG - 1),
                                )
                    if t < n:
                        h0n = statep.tile([128, 2 * BL], BF16, tag="h0")
                        c0 = gates(z0_t.pop(t), c0, "a", h0n[:])
                        st["h0_hist"][t] = h0n
                        if t - 2 in st["h0_hist"]:
                            del st["h0_hist"][t - 2]
                        h0T = h0n
                    if z1 is not None:
                        tp = t - 1
                        c1 = gates(z1, c1, "b",
                                   outT[:, tp * 2 * BL:(tp + 1) * 2 * BL])
                st["h0T"], st["c0"], st["c1"] = h0T, c0, c1
                st["h1"] = outT[:, (n - 1) * 2 * BL:n * 2 * BL]
                return st

            with (
                tc.tile_pool(name="psA", bufs=2, space="PSUM") as psA,
                tc.tile_pool(name="psB", bufs=2, space="PSUM") as psB,
            ):
                st = zeros_state()
                st = recurrence(w0e_sb, w1e_sb, b1e_sb, gxe_d, ns, encT, st,
                                psA, psB)
                st = recurrence(w0d_sb, w1d_sb, b1d_sb, gxd_d, nt, decT, st,
                                psA, psB)

            # ---------------- attention -------------------------------------
            nsc = ns // 128
            ntc = nt // 128
            with (
                tc.tile_pool(name="attn", bufs=2) as attp,
                tc.tile_pool(name="attnc", bufs=1) as attc,
                tc.tile_pool(name="attps", bufs=2, space="PSUM") as attps,
            ):
                enc_nat = attc.tile([128, BL, nsc, H], F32)
                encT4 = encT[:].rearrange("p (s k b) -> p s k b", k=KH, b=BL)
                decT4 = decT[:].rearrange("p (s k b) -> p s k b", k=KH, b=BL)
                for b in range(BL):
                    for k in range(KH):
                        for sc in range(nsc):
                            pt = attps.tile([128, 128], BF16, tag="ptb")
                            nc.tensor.transpose(
                                pt[:], encT4[:, sc * 128:(sc + 1) * 128, k, b],
                                identb_sb[:],
                            )
                            nc.scalar.copy(
                                enc_nat[:, b, sc, k * 128:(k + 1) * 128], pt[:]
                            )
                    for tcn in range(ntc):
                        ps_s = attps.tile([128, ns], F32, tag="ps_s")
                        for k in range(KH):
                            nc.tensor.matmul(
                                ps_s[:],
                                decT4[:, tcn * 128:(tcn + 1) * 128, k, b],
                                encT4[:, :, k, b],
                                start=(k == 0), stop=(k == KH - 1),
                            )
                        nmx = attp.tile([128, 1], F32, tag="nmx")
                        nc.vector.tensor_reduce(
                            nmx[:], ps_s[:], axis=AX.X, op=ALU.max, negate=True
                        )
                        wexp = attp.tile([128, ns], F32, tag="wexp")
                        den = attp.tile([128, 1], F32, tag="den")
                        nc.scalar.activation(
                            wexp[:], ps_s[:], AF.Exp, bias=nmx[:],
                            accum_out=den[:],
                        )
                        rden = attp.tile([128, 1], F32, tag="rden")
                        nc.vector.reciprocal(rden[:], den[:])
                        wn = attp.tile([128, ns], F32, tag="wn")
                        nc.vector.tensor_scalar_mul(wn[:], wexp[:], rden[:])
                        nc.sync.dma_start(
                            attnw[b, tcn * 128:(tcn + 1) * 128, :], wn[:]
                        )
                        wT = attp.tile([128, nsc * 128], F32, tag="wT")
                        for j in range(nsc):
                            ptw = attps.tile([128, 128], F32, tag="pt")
                            nc.tensor.transpose(
                                ptw[:], wn[:, j * 128:(j + 1) * 128], ident_sb[:]
                            )
                            nc.scalar.copy(wT[:, j * 128:(j + 1) * 128], ptw[:])
                        ps_v = attps.tile([128, H], F32, tag="ps_v")
                        for j in range(nsc):
                            nc.tensor.matmul(
                                ps_v[:],
                                wT[:, j * 128:(j + 1) * 128],
                                enc_nat[:, b, j, :],
                                start=(j == 0), stop=(j == nsc - 1),
                            )
                        vsb = attp.tile([128, H], F32, tag="vsb")
                        nc.scalar.copy(vsb[:], ps_v[:])
                        nc.sync.dma_start(
                            attnv[b, tcn * 128:(tcn + 1) * 128, :], vsb[:]
                        )
                nc.sync.dma_start(dect[:], decT[:])
    nc.compile()
    return nc


# ---------------------- host-side layout helpers ----------------------------

def _prep_xt(x):
    """(n, BL, 256) f32 -> (128, 2*n*BL) bf16, col = k*(n*BL) + t*BL + b."""
    n = x.shape[0]
    a = np.ascontiguousarray(x.transpose(2, 0, 1)).reshape(KH, 128, n * BL)
    return np.concatenate([a[0], a[1]], axis=1).astype(ml_dtypes.bfloat16)


def _prep_lhsT(Wp):
    """Permuted weight (1024, Kdim) -> (128, KT*8*128) bf16 lhsT tiles,
    col block j = m*KT + k."""
    Kd = Wp.shape[1]
    KT = Kd // 128
    t4 = np.ascontiguousarray(Wp.T).reshape(KT, 128, M8, 128)
    return np.ascontiguousarray(
        t4.transpose(1, 2, 0, 3)
    ).reshape(128, M8 * KT * 128).astype(ml_dtypes.bfloat16)


def _prep_shared(inputs, ns, nt):
    f = lambda k: np.asarray(inputs[k], np.float32)
    sh = {}
    for tag, wih, whh, bih, bhh in (
        ("e", f("enc_Wih"), f("enc_Whh"), f("enc_bih"), f("enc_bhh")),
        ("d", f("dec_Wih"), f("dec_Whh"), f("dec_bih"), f("dec_bhh")),
    ):
        sh["w0" + tag] = _prep_lhsT(whh[0][GATE_ORDER])
        sh["w1" + tag] = _prep_lhsT(
            np.concatenate([wih[1], whh[1]], axis=1)[GATE_ORDER]
        )
        sh["wi" + tag] = _prep_lhsT(wih[0][GATE_ORDER])
        b0 = (bih[0] + bhh[0])[GATE_ORDER]
        sh["b0" + tag] = np.ascontiguousarray(b0.reshape(M8, 128).T)
        b1 = (bih[1] + bhh[1])[GATE_ORDER]
        b1r = b1.reshape(M8, 128).T          # (128, 8)
        sh["b1" + tag] = np.ascontiguousarray(
            np.repeat(b1r[:, :, None], BL, axis=2).reshape(128, M8 * BL)
        ).astype(ml_dtypes.bfloat16)
    sh["ident"] = np.eye(128, dtype=np.float32)
    sh["identb"] = np.eye(128, dtype=ml_dtypes.bfloat16)
    return sh


_BUILT = {}


def _get_nc(ns, nt):
    key = (ns, nt)
    if key not in _BUILT:
        _BUILT[key] = build_nc(ns, nt)
    return _BUILT[key]


def run(inputs, ns=S, nt=T):
    """Run the kernel; returns (responses, attn_w) full-shape."""
    nc = _get_nc(ns, nt)
    enc_in = np.asarray(inputs["enc_input"], np.float32)[:ns]
    dec_in = np.asarray(inputs["dec_input"], np.float32)[:nt]
    nb = enc_in.shape[1]
    ncores = nb // BL
    shared = _prep_shared(inputs, ns, nt)
    in_maps = []
    for c in range(ncores):
        m = dict(shared)
        sl = slice(c * BL, (c + 1) * BL)
        m["xte"] = _prep_xt(enc_in[:, sl, :])
        m["xtd"] = _prep_xt(dec_in[:, sl, :])
        in_maps.append(m)
    res = run_bass_kernel_spmd(nc, in_maps, list(range(ncores)))
    resp = np.empty((nt, nb, 2 * H), np.float32)
    attw = np.empty((nt, nb, ns), np.float32)
    for c in range(ncores):
        r = res.results[c]
        sl = slice(c * BL, (c + 1) * BL)
        dect = r["dect"].astype(np.float32).reshape(128, nt, KH, BL)
        resp[:, sl, 0:H] = np.ascontiguousarray(
            dect.transpose(1, 3, 2, 0)
        ).reshape(nt, BL, H)
        resp[:, sl, H:2 * H] = r["attnv"].transpose(1, 0, 2)
        attw[:, sl, :] = r["attnw"].transpose(1, 0, 2)
    return resp, attw


def kernel(**inputs):
    return run(inputs, S, T)


# revision 16
# speedup vs baseline: 2816.2882x; 1.5064x over previous
"""AttentionLSTM (2-layer enc/dec LSTM + dot-product attention) on 8 trn2 NeuronCores.

Sharding: data-parallel over batch (B=64 -> 8 cores x 8). Per core:
  - On-chip layout is feature-major: hidden state h kept as (h, b) tiles so the
    recurrent matmuls (gates-stationary, bf16 weights) need no transposes.
  - Input projections gx = Wih @ x^T + b precomputed in bulk -> DRAM, streamed
    back per step.
  - Encoder 512 steps -> encT; decoder 512 steps -> decT (wavefront across the
    two layers); then attention (scores/softmax/attn_v) per batch element.
Host does all weight/input layout preprocessing and output reassembly.
"""
import sys

import numpy as np

for _p in ("/opt/trn_rl_repo", "/root/.axon_site/_ro/trn_rl_repo"):
    if _p not in sys.path:
        sys.path.append(_p)

import ml_dtypes  # noqa: E402
import concourse.bass as bass  # noqa: E402
import concourse.bacc as bacc  # noqa: E402
import concourse.mybir as mybir  # noqa: E402
from concourse import tile  # noqa: E402
from concourse.bass_utils import run_bass_kernel_spmd  # noqa: E402

F32 = mybir.dt.float32
BF16 = mybir.dt.bfloat16
AF = mybir.ActivationFunctionType
ALU = mybir.AluOpType
AX = mybir.AxisListType

NCORES = 8
S, T, B = 512, 512, 64
BL = B // NCORES          # 8 batch per core
H = 256
IN = 256
G = 4 * H                 # 1024 gates
KH = H // 128             # 2 k-tiles for hidden contraction
KHG = (H + H) // 128      # 4 k-tiles for layer-1 contraction [h0; h1]
M8 = G // 128             # 8 gate m-tiles
CH = 8                    # recurrence steps per gx DMA chunk

# torch gate order (i, f, g, o) -> on-chip order (i, f, o, g) so one sigmoid
# covers cols [0, 6*BL) and one tanh covers [6*BL, 8*BL)
GATE_ORDER = np.concatenate(
    [np.arange(0, 2 * H), np.arange(3 * H, 4 * H), np.arange(2 * H, 3 * H)]
)


def build_nc(ns=S, nt=T, reps=1):
    nc = bacc.Bacc("TRN2", target_bir_lowering=False, debug=False,
                   num_devices=NCORES)

    def inp(name, shape, dt):
        return nc.dram_tensor(name, list(shape), dt, kind="ExternalInput")

    xte = inp("xte", (128, KH * ns * BL), BF16)      # col = k*(ns*BL) + t*BL + b
    xtd = inp("xtd", (128, KH * nt * BL), BF16)
    w0e = inp("w0e", (128, KH * M8 * 128), BF16)     # lhsT tiles, col j = m*KH + k
    w1e = inp("w1e", (128, KHG * M8 * 128), BF16)    # j = m*KHG + k
    w0d = inp("w0d", (128, KH * M8 * 128), BF16)
    w1d = inp("w1d", (128, KHG * M8 * 128), BF16)
    wie = inp("wie", (128, KH * M8 * 128), BF16)     # Wih0^T tiles for prologue
    wid = inp("wid", (128, KH * M8 * 128), BF16)
    b0e = inp("b0e", (128, M8), F32)
    b0d = inp("b0d", (128, M8), F32)
    b1e = inp("b1e", (128, M8 * BL), BF16)
    b1d = inp("b1d", (128, M8 * BL), BF16)
    ident = inp("ident", (128, 128), F32)
    identb = inp("identb", (128, 128), BF16)

    dect = nc.dram_tensor("dect", [128, nt * 2 * BL], BF16, kind="ExternalOutput")
    attnv = nc.dram_tensor("attnv", [BL, nt, H], F32, kind="ExternalOutput")
    attnw = nc.dram_tensor("attnw", [BL, nt, ns], F32, kind="ExternalOutput")

    with tile.TileContext(nc) as tc:
        with (
            tc.tile_pool(name="const", bufs=1) as constp,
            tc.tile_pool(name="store", bufs=1) as storep,
            tc.tile_pool(name="dram", bufs=1, space="DRAM") as dramp,
            tc.tile_pool(name="state", bufs=2) as statep,
            tc.tile_pool(name="gxp", bufs=4) as gxp,
            tc.tile_pool(name="work", bufs=3) as workp,
        ):
            def load_const(dram_t, shape, dt):
                t = constp.tile(shape, dt, name=dram_t.name + "_sb")
                nc.sync.dma_start(t[:], dram_t[:])
                return t

            w0e_sb = load_const(w0e, [128, KH * M8 * 128], BF16)
            w1e_sb = load_const(w1e, [128, KHG * M8 * 128], BF16)
            w0d_sb = load_const(w0d, [128, KH * M8 * 128], BF16)
            w1d_sb = load_const(w1d, [128, KHG * M8 * 128], BF16)
            wie_sb = load_const(wie, [128, KH * M8 * 128], BF16)
            wid_sb = load_const(wid, [128, KH * M8 * 128], BF16)
            b0e_sb = load_const(b0e, [128, M8], F32)
            b0d_sb = load_const(b0d, [128, M8], F32)
            b1e_sb = load_const(b1e, [128, M8 * BL], BF16)
            b1d_sb = load_const(b1d, [128, M8 * BL], BF16)
            ident_sb = load_const(ident, [128, 128], F32)
            identb_sb = load_const(identb, [128, 128], BF16)

            encT = storep.tile([128, ns * 2 * BL], BF16)  # col = t*16 + k*8 + b
            decT = storep.tile([128, nt * 2 * BL], BF16)

            gxe_d = dramp.tile([128, ns, M8 * BL], BF16)
            gxd_d = dramp.tile([128, nt, M8 * BL], BF16)

            # ---------------- prologue: gx = Wih0 @ x^T + b0 -> DRAM ----------
            with (
                tc.tile_pool(name="prolog", bufs=2) as prop,
                tc.tile_pool(name="propsum", bufs=4, space="PSUM") as props,
            ):
                for xt_dram, wi_sb, b0_sb, gx_d, n in (
                    (xte, wie_sb, b0e_sb, gxe_d, ns),
                    (xtd, wid_sb, b0d_sb, gxd_d, nt),
                ):
                    xt_sb = prop.tile([128, KH * n * BL], BF16, tag="xt")
                    nc.sync.dma_start(xt_sb[:], xt_dram[:])
                    nchunks = (n * BL) // 512    # 64 steps per chunk
                    tpc = 512 // BL              # steps per chunk
                    for c in range(nchunks):
                        sg = prop.tile([128, tpc, M8 * BL], BF16, tag="sg")
                        for m in range(M8):
                            ps = props.tile([128, 512], F32, tag="pp")
                            for k in range(KH):
                                j = m * KH + k
                                nc.tensor.matmul(
                                    ps[:],
                                    wi_sb[:, j * 128:(j + 1) * 128],
                                    xt_sb[:, k * n * BL + c * 512:
                                          k * n * BL + (c + 1) * 512],
                                    start=(k == 0), stop=(k == KH - 1),
                                )
                            ps3 = ps[:].rearrange("p (t b) -> p t b", b=BL)
                            dst = sg[:, :, m * BL:(m + 1) * BL]
                            if m % 2 == 0:
                                nc.scalar.activation(
                                    dst, ps3, AF.Identity, bias=b0_sb[:, m:m + 1]
                                )
                            else:
                                nc.vector.tensor_scalar_add(
                                    dst, ps3, b0_sb[:, m:m + 1]
                                )
                        nc.sync.dma_start(gx_d[:, c * tpc:(c + 1) * tpc, :], sg[:])

            # ---------------- recurrences ------------------------------------
            def zeros_state():
                h0 = statep.tile([128, KH * BL], BF16, tag="h0")
                c0 = statep.tile([128, KH * BL], F32, tag="ac")
                h1 = statep.tile([128, KH * BL], BF16, tag="h1z")
                c1 = statep.tile([128, KH * BL], F32, tag="bc")
                for t_ in (h0, c0, h1, c1):
                    nc.gpsimd.memset(t_[:], 0.0)
                return {"h0T": h0, "c0": c0, "h1": h1[:], "c1": c1,
                        "h0_hist": {-1: h0}}

            def gates(z, c_prev, tag, h_out):
                """z (128, 8*BL) PSUM pre-activations [i f o g] -> writes
                h into h_out AP, returns c_new."""
                s = workp.tile([128, 6 * BL], F32, tag=tag + "s")
                nc.scalar.activation(s[:], z[:, 0:6 * BL], AF.Sigmoid)
                g = workp.tile([128, 2 * BL], F32, tag=tag + "g")
                nc.scalar.activation(g[:], z[:, 6 * BL:8 * BL], AF.Tanh)
                t1 = workp.tile([128, 2 * BL], F32, tag=tag + "t1")
                nc.vector.tensor_mul(t1[:], s[:, 0:2 * BL], g[:])
                u = workp.tile([128, 2 * BL], F32, tag=tag + "u")
                nc.vector.tensor_mul(u[:], s[:, 2 * BL:4 * BL], c_prev[:])
                c_new = statep.tile([128, 2 * BL], F32, tag=tag + "c")
                nc.vector.tensor_add(c_new[:], u[:], t1[:])
                tch = workp.tile([128, 2 * BL], F32, tag=tag + "tc")
                nc.scalar.activation(tch[:], c_new[:], AF.Tanh)
                nc.vector.tensor_mul(h_out, s[:, 4 * BL:6 * BL], tch[:])
                return c_new

            def recurrence(w0_sb, w1_sb, b1_sb, gx_d, n, outT, st, psA, psB):
                """Wavefront: iteration t emits L0 step t and L1 step t-1 so
                the PE always has independent work while L0's elementwise
                chain completes.  h1 state lives directly in outT (bf16)."""
                h0T, c0, c1 = st["h0T"], st["c0"], st["c1"]
                h1_prev = st["h1"]      # AP (128, 16) bf16 (prev LSTM's final)
                nch = (n + CH - 1) // CH
                gx_tiles = {}

                def issue_gx(c):
                    if c < nch:
                        gt = gxp.tile([128, CH, M8 * BL], BF16, tag="gx")
                        nc.sync.dma_start(gt[:], gx_d[:, c * CH:(c + 1) * CH, :])
                        gx_tiles[c] = gt

                for c in range(min(3, nch)):
                    issue_gx(c)

                z0_t = {}
                for t in range(n + 1):
                    z1 = None
                    if t < n:
                        if t % CH == 0:
                            issue_gx(t // CH + 3)
                        gxt = gx_tiles[t // CH][:, t % CH, :]
                        # L0 matmuls: gx preload via identity, then weights
                        z0 = psA.tile([128, M8 * BL], F32, tag="z0")
                        nc.tensor.matmul(z0[:], identb_sb[:], gxt,
                                         start=True, stop=False)
                        for m in range(M8):
                            for k in range(KH):
                                j = m * KH + k
                                nc.tensor.matmul(
                                    z0[:, m * BL:(m + 1) * BL],
                                    w0_sb[:, j * 128:(j + 1) * 128],
                                    h0T[:, k * BL:(k + 1) * BL],
                                    start=False,
                                    stop=(m == M8 - 1 and k == KH - 1),
                                )
                        z0_t[t] = z0
                    if t >= 1:
                        # L1 matmuls for step t-1; h1-reading MMs (k>=KH)
                        # emitted last to relax the h1 deadline.
                        tp = t - 1
                        h0_in = st["h0_hist"][tp]
                        h1_in = (outT[:, (tp - 1) * 2 * BL:tp * 2 * BL]
                                 if tp >= 1 else h1_prev)
                        z1 = psB.tile([128, M8 * BL], F32, tag="z1")
                        nc.tensor.matmul(z1[:], identb_sb[:], b1_sb[:],
                                         start=True, stop=False)
                        for k in range(KHG):
                            rhs_all = (h0_in[:, k * BL:(k + 1) * BL] if k < KH
                                       else h1_in[:, (k - KH) * BL:
                                                  (k - KH + 1) * BL])
                            for m in range(M8):
                                j = m * KHG + k
                                nc.tensor.matmul(
                                    z1[:, m * BL:(m + 1) * BL],
                                    w1_sb[:, j * 128:(j + 1) * 128],
                                    rhs_all,
                                    start=False,
                                    stop=(m == M8 - 1 and k == KHG - 1),
                                )
                    if t < n:
                        h0n = statep.tile([128, 2 * BL], BF16, tag="h0")
                        c0 = gates(z0_t.pop(t), c0, "a", h0n[:])
                        st["h0_hist"][t] = h0n
                        if t - 2 in st["h0_hist"]:
                            del st["h0_hist"][t - 2]
                        h0T = h0n
                    if z1 is not None:
                        tp = t - 1
                        c1 = gates(z1, c1, "b",
                                   outT[:, tp * 2 * BL:(tp + 1) * 2 * BL])
                st["h0T"], st["c0"], st["c1"] = h0T, c0, c1
                st["h1"] = outT[:, (n - 1) * 2 * BL:n * 2 * BL]
                return st

            with (
                tc.tile_pool(name="psA", bufs=2, space="PSUM") as psA,
                tc.tile_pool(name="psB", bufs=2, space="PSUM") as psB,
            ):
                st = zeros_state()
                st = recurrence(w0e_sb, w1e_sb, b1e_sb, gxe_d, ns, encT, st,
                                psA, psB)
                st = recurrence(w0d_sb, w1d_sb, b1d_sb, gxd_d, nt, decT, st,
                                psA, psB)
                # timing-only extra repetitions of the steady-state loop;
                # chained through the live state so nothing is dead-code
                # eliminated (encT/decT rewritten identically afterwards).
                for _ in range(reps - 1):
                    st = recurrence(w0e_sb, w1e_sb, b1e_sb, gxe_d, ns, encT,
                                    st, psA, psB)
                    st = recurrence(w0d_sb, w1d_sb, b1d_sb, gxd_d, nt, decT,
                                    st, psA, psB)

            # ---------------- attention -------------------------------------
            nsc = ns // 128
            ntc = nt // 128
            with (
                tc.tile_pool(name="attn", bufs=2) as attp,
                tc.tile_pool(name="attnc", bufs=1) as attc,
                tc.tile_pool(name="attps", bufs=2, space="PSUM") as attps,
            ):
                enc_nat = attc.tile([128, BL, nsc, H], F32)
                encT4 = encT[:].rearrange("p (s k b) -> p s k b", k=KH, b=BL)
                decT4 = decT[:].rearrange("p (s k b) -> p s k b", k=KH, b=BL)
                for b in range(BL):
                    for k in range(KH):
                        for sc in range(nsc):
                            pt = attps.tile([128, 128], BF16, tag="ptb")
                            nc.tensor.transpose(
                                pt[:], encT4[:, sc * 128:(sc + 1) * 128, k, b],
                                identb_sb[:],
                            )
                            nc.scalar.copy(
                                enc_nat[:, b, sc, k * 128:(k + 1) * 128], pt[:]
                            )
                    for tcn in range(ntc):
                        ps_s = attps.tile([128, ns], F32, tag="ps_s")
                        for k in range(KH):
                            nc.tensor.matmul(
                                ps_s[:],
                                decT4[:, tcn * 128:(tcn + 1) * 128, k, b],
                                encT4[:, :, k, b],
                                start=(k == 0), stop=(k == KH - 1),
                            )
                        nmx = attp.tile([128, 1], F32, tag="nmx")
                        nc.vector.tensor_reduce(
                            nmx[:], ps_s[:], axis=AX.X, op=ALU.max, negate=True
                        )
                        wexp = attp.tile([128, ns], F32, tag="wexp")
                        den = attp.tile([128, 1], F32, tag="den")
                        nc.scalar.activation(
                            wexp[:], ps_s[:], AF.Exp, bias=nmx[:],
                            accum_out=den[:],
                        )
                        rden = attp.tile([128, 1], F32, tag="rden")
                        nc.vector.reciprocal(rden[:], den[:])
                        wn = attp.tile([128, ns], F32, tag="wn")
                        nc.vector.tensor_scalar_mul(wn[:], wexp[:], rden[:])
                        nc.sync.dma_start(
                            attnw[b, tcn * 128:(tcn + 1) * 128, :], wn[:]
                        )
                        wT = attp.tile([128, nsc * 128], F32, tag="wT")
                        for j in range(nsc):
                            ptw = attps.tile([128, 128], F32, tag="pt")
                            nc.tensor.transpose(
                                ptw[:], wn[:, j * 128:(j + 1) * 128], ident_sb[:]
                            )
                            nc.scalar.copy(wT[:, j * 128:(j + 1) * 128], ptw[:])
                        ps_v = attps.tile([128, H], F32, tag="ps_v")
                        for j in range(nsc):
                            nc.tensor.matmul(
                                ps_v[:],
                                wT[:, j * 128:(j + 1) * 128],
                                enc_nat[:, b, j, :],
                                start=(j == 0), stop=(j == nsc - 1),
                            )
                        vsb = attp.tile([128, H], F32, tag="vsb")
                        nc.scalar.copy(vsb[:], ps_v[:])
                        nc.sync.dma_start(
                            attnv[b, tcn * 128:(tcn + 1) * 128, :], vsb[:]
                        )
                nc.sync.dma_start(dect[:], decT[:])
    nc.compile()
    return nc


# ---------------------- host-side layout helpers ----------------------------

def _prep_xt(x):
    """(n, BL, 256) f32 -> (128, 2*n*BL) bf16, col = k*(n*BL) + t*BL + b."""
    n = x.shape[0]
    a = np.ascontiguousarray(x.transpose(2, 0, 1)).reshape(KH, 128, n * BL)
    return np.concatenate([a[0], a[1]], axis=1).astype(ml_dtypes.bfloat16)


def _prep_lhsT(Wp):
    """Permuted weight (1024, Kdim) -> (128, KT*8*128) bf16 lhsT tiles,
    col block j = m*KT + k."""
    Kd = Wp.shape[1]
    KT = Kd // 128
    t4 = np.ascontiguousarray(Wp.T).reshape(KT, 128, M8, 128)
    return np.ascontiguousarray(
        t4.transpose(1, 2, 0, 3)
    ).reshape(128, M8 * KT * 128).astype(ml_dtypes.bfloat16)


def _prep_shared(inputs, ns, nt):
    f = lambda k: np.asarray(inputs[k], np.float32)
    sh = {}
    for tag, wih, whh, bih, bhh in (
        ("e", f("enc_Wih"), f("enc_Whh"), f("enc_bih"), f("enc_bhh")),
        ("d", f("dec_Wih"), f("dec_Whh"), f("dec_bih"), f("dec_bhh")),
    ):
        sh["w0" + tag] = _prep_lhsT(whh[0][GATE_ORDER])
        sh["w1" + tag] = _prep_lhsT(
            np.concatenate([wih[1], whh[1]], axis=1)[GATE_ORDER]
        )
        sh["wi" + tag] = _prep_lhsT(wih[0][GATE_ORDER])
        b0 = (bih[0] + bhh[0])[GATE_ORDER]
        sh["b0" + tag] = np.ascontiguousarray(b0.reshape(M8, 128).T)
        b1 = (bih[1] + bhh[1])[GATE_ORDER]
        b1r = b1.reshape(M8, 128).T          # (128, 8)
        sh["b1" + tag] = np.ascontiguousarray(
            np.repeat(b1r[:, :, None], BL, axis=2).reshape(128, M8 * BL)
        ).astype(ml_dtypes.bfloat16)
    sh["ident"] = np.eye(128, dtype=np.float32)
    sh["identb"] = np.eye(128, dtype=ml_dtypes.bfloat16)
    return sh


_BUILT = {}


def _get_nc(ns, nt):
    key = (ns, nt)
    if key not in _BUILT:
        _BUILT[key] = build_nc(ns, nt)
    return _BUILT[key]


def run(inputs, ns=S, nt=T):
    """Run the kernel; returns (responses, attn_w) full-shape."""
    nc = _get_nc(ns, nt)
    enc_in = np.asarray(inputs["enc_input"], np.float32)[:ns]
    dec_in = np.asarray(inputs["dec_input"], np.float32)[:nt]
    nb = enc_in.shape[1]
    ncores = nb // BL
    shared = _prep_shared(inputs, ns, nt)
    in_maps = []
    for c in range(ncores):
        m = dict(shared)
        sl = slice(c * BL, (c + 1) * BL)
        m["xte"] = _prep_xt(enc_in[:, sl, :])
        m["xtd"] = _prep_xt(dec_in[:, sl, :])
        in_maps.append(m)
    res = run_bass_kernel_spmd(nc, in_maps, list(range(ncores)))
    resp = np.empty((nt, nb, 2 * H), np.float32)
    attw = np.empty((nt, nb, ns), np.float32)
    for c in range(ncores):
        r = res.results[c]
        sl = slice(c * BL, (c + 1) * BL)
        dect = r["dect"].astype(np.float32).reshape(128, nt, KH, BL)
        resp[:, sl, 0:H] = np.ascontiguousarray(
            dect.transpose(1, 3, 2, 0)
        ).reshape(nt, BL, H)
        resp[:, sl, H:2 * H] = r["attnv"].transpose(1, 0, 2)
        attw[:, sl, :] = r["attnw"].transpose(1, 0, 2)
    return resp, attw


def kernel(**inputs):
    return run(inputs, S, T)


# revision 24
# speedup vs baseline: 2821.5646x; 1.0019x over previous
"""AttentionLSTM (2-layer enc/dec LSTM + dot-product attention) on 8 trn2 NeuronCores.

Sharding: data-parallel over batch (B=64 -> 8 cores x 8). Per core:
  - On-chip layout is feature-major: hidden state h kept as (h, b) tiles so the
    recurrent matmuls (gates-stationary, bf16 weights) need no transposes.
  - Input projections gx = Wih @ x^T + b precomputed in bulk -> DRAM, streamed
    back per step.
  - Encoder 512 steps -> encT; decoder 512 steps -> decT (wavefront across the
    two layers); then attention (scores/softmax/attn_v) per batch element.
Host does all weight/input layout preprocessing and output reassembly.
"""
import sys

import numpy as np

for _p in ("/opt/trn_rl_repo", "/root/.axon_site/_ro/trn_rl_repo"):
    if _p not in sys.path:
        sys.path.append(_p)

import ml_dtypes  # noqa: E402
import concourse.bass as bass  # noqa: E402
import concourse.bacc as bacc  # noqa: E402
import concourse.mybir as mybir  # noqa: E402
from concourse import tile  # noqa: E402
from concourse.bass_utils import run_bass_kernel_spmd  # noqa: E402

F32 = mybir.dt.float32
BF16 = mybir.dt.bfloat16
AF = mybir.ActivationFunctionType
ALU = mybir.AluOpType
AX = mybir.AxisListType

NCORES = 8
S, T, B = 512, 512, 64
BL = B // NCORES          # 8 batch per core
H = 256
IN = 256
G = 4 * H                 # 1024 gates
KH = H // 128             # 2 k-tiles for hidden contraction
KHG = (H + H) // 128      # 4 k-tiles for layer-1 contraction [h0; h1]
M8 = G // 128             # 8 gate m-tiles
CH = 8                    # recurrence steps per gx DMA chunk

# torch gate order (i, f, g, o) -> on-chip order (i, f, o, g) so one sigmoid
# covers cols [0, 6*BL) and one tanh covers [6*BL, 8*BL)
GATE_ORDER = np.concatenate(
    [np.arange(0, 2 * H), np.arange(3 * H, 4 * H), np.arange(2 * H, 3 * H)]
)


def build_nc(ns=S, nt=T, reps=1):
    nc = bacc.Bacc("TRN2", target_bir_lowering=False, debug=False,
                   num_devices=NCORES)

    def inp(name, shape, dt):
        return nc.dram_tensor(name, list(shape), dt, kind="ExternalInput")

    xte = inp("xte", (128, KH * ns * BL), BF16)      # col = k*(ns*BL) + t*BL + b
    xtd = inp("xtd", (128, KH * nt * BL), BF16)
    w0e = inp("w0e", (128, KH * M8 * 128), BF16)     # lhsT tiles, col j = m*KH + k
    w1e = inp("w1e", (128, KHG * M8 * 128), BF16)    # j = m*KHG + k
    w0d = inp("w0d", (128, KH * M8 * 128), BF16)
    w1d = inp("w1d", (128, KHG * M8 * 128), BF16)
    wie = inp("wie", (128, KH * M8 * 128), BF16)     # Wih0^T tiles for prologue
    wid = inp("wid", (128, KH * M8 * 128), BF16)
    b0e = inp("b0e", (128, M8), F32)
    b0d = inp("b0d", (128, M8), F32)
    b1e = inp("b1e", (128, M8 * BL), BF16)
    b1d = inp("b1d", (128, M8 * BL), BF16)
    ident = inp("ident", (128, 128), F32)
    identb = inp("identb", (128, 128), BF16)

    dect = nc.dram_tensor("dect", [128, nt * 2 * BL], BF16, kind="ExternalOutput")
    attnv = nc.dram_tensor("attnv", [BL, nt, H], F32, kind="ExternalOutput")
    attnw = nc.dram_tensor("attnw", [BL, nt, ns], F32, kind="ExternalOutput")

    with tile.TileContext(nc) as tc:
        with (
            tc.tile_pool(name="const", bufs=1) as constp,
            tc.tile_pool(name="store", bufs=1) as storep,
            tc.tile_pool(name="dram", bufs=1, space="DRAM") as dramp,
            tc.tile_pool(name="state", bufs=2) as statep,
            tc.tile_pool(name="gxp", bufs=4) as gxp,
            tc.tile_pool(name="work", bufs=3) as workp,
        ):
            def load_const(dram_t, shape, dt):
                t = constp.tile(shape, dt, name=dram_t.name + "_sb")
                nc.sync.dma_start(t[:], dram_t[:])
                return t

            w0e_sb = load_const(w0e, [128, KH * M8 * 128], BF16)
            w1e_sb = load_const(w1e, [128, KHG * M8 * 128], BF16)
            w0d_sb = load_const(w0d, [128, KH * M8 * 128], BF16)
            w1d_sb = load_const(w1d, [128, KHG * M8 * 128], BF16)
            wie_sb = load_const(wie, [128, KH * M8 * 128], BF16)
            wid_sb = load_const(wid, [128, KH * M8 * 128], BF16)
            b0e_sb = load_const(b0e, [128, M8], F32)
            b0d_sb = load_const(b0d, [128, M8], F32)
            b1e_sb = load_const(b1e, [128, M8 * BL], BF16)
            b1d_sb = load_const(b1d, [128, M8 * BL], BF16)
            ident_sb = load_const(ident, [128, 128], F32)
            identb_sb = load_const(identb, [128, 128], BF16)

            encT = storep.tile([128, ns * 2 * BL], BF16)  # col = t*16 + k*8 + b
            decT = storep.tile([128, nt * 2 * BL], BF16)

            gxe_d = dramp.tile([128, ns, M8 * BL], BF16)
            gxd_d = dramp.tile([128, nt, M8 * BL], BF16)

            # ---------------- prologue: gx = Wih0 @ x^T + b0 -> DRAM ----------
            with (
                tc.tile_pool(name="prolog", bufs=2) as prop,
                tc.tile_pool(name="propsum", bufs=4, space="PSUM") as props,
            ):
                for xt_dram, wi_sb, b0_sb, gx_d, n in (
                    (xte, wie_sb, b0e_sb, gxe_d, ns),
                    (xtd, wid_sb, b0d_sb, gxd_d, nt),
                ):
                    xt_sb = prop.tile([128, KH * n * BL], BF16, tag="xt")
                    nc.sync.dma_start(xt_sb[:], xt_dram[:])
                    nchunks = (n * BL) // 512    # 64 steps per chunk
                    tpc = 512 // BL              # steps per chunk
                    for c in range(nchunks):
                        sg = prop.tile([128, tpc, M8 * BL], BF16, tag="sg")
                        for m in range(M8):
                            ps = props.tile([128, 512], F32, tag="pp")
                            for k in range(KH):
                                j = m * KH + k
                                nc.tensor.matmul(
                                    ps[:],
                                    wi_sb[:, j * 128:(j + 1) * 128],
                                    xt_sb[:, k * n * BL + c * 512:
                                          k * n * BL + (c + 1) * 512],
                                    start=(k == 0), stop=(k == KH - 1),
                                )
                            ps3 = ps[:].rearrange("p (t b) -> p t b", b=BL)
                            dst = sg[:, :, m * BL:(m + 1) * BL]
                            if m % 2 == 0:
                                nc.scalar.activation(
                                    dst, ps3, AF.Identity, bias=b0_sb[:, m:m + 1]
                                )
                            else:
                                nc.vector.tensor_scalar_add(
                                    dst, ps3, b0_sb[:, m:m + 1]
                                )
                        nc.sync.dma_start(gx_d[:, c * tpc:(c + 1) * tpc, :], sg[:])

            # ---------------- recurrences ------------------------------------
            NCH = 2                     # independent batch chains per core
            BC = BL // NCH              # batch per chain

            def zeros_state(cn):
                p = f"q{cn}"
                h0 = statep.tile([128, KH * BC], BF16, tag=p + "h0",
                                 name=p + "h0z")
                c0 = statep.tile([128, KH * BC], F32, tag=p + "ac",
                                 name=p + "c0z")
                h1 = statep.tile([128, KH * BC], BF16, tag=p + "h1z",
                                 name=p + "h1z")
                c1 = statep.tile([128, KH * BC], F32, tag=p + "bc",
                                 name=p + "c1z")
                for t_ in (h0, c0, h1, c1):
                    nc.gpsimd.memset(t_[:], 0.0)
                return {"h0T": h0, "c0": c0, "c1": c1,
                        "h1k": [h1[:, kk * BC:(kk + 1) * BC]
                                for kk in range(KH)],
                        "h0_hist": {-1: h0}}

            def gates(z, c_prev, tag, h_out):
                """z (128, 8*BC) PSUM pre-activations [i f o 2g] -> writes
                h into h_out AP, returns c_new.  The g-gate weights are
                pre-scaled by 2 on host so tanh(g) = 2*sigmoid(2g) - 1
                comes out of one wide sigmoid + a DVE affine."""
                s = workp.tile([128, 8 * BC], F32, tag=tag + "s")
                nc.scalar.activation(s[:], z[:], AF.Sigmoid)
                g = workp.tile([128, 2 * BC], F32, tag=tag + "g")
                nc.vector.tensor_scalar(g[:], s[:, 6 * BC:8 * BC], 2.0, -1.0,
                                        ALU.mult, ALU.add)
                t1 = workp.tile([128, 2 * BC], F32, tag=tag + "t1")
                nc.vector.tensor_mul(t1[:], s[:, 0:2 * BC], g[:])
                u = workp.tile([128, 2 * BC], F32, tag=tag + "u")
                nc.vector.tensor_mul(u[:], s[:, 2 * BC:4 * BC], c_prev[:])
                c_new = statep.tile([128, 2 * BC], F32, tag=tag + "c")
                nc.vector.tensor_add(c_new[:], u[:], t1[:])
                tch = workp.tile([128, 2 * BC], F32, tag=tag + "tc")
                nc.scalar.activation(tch[:], c_new[:], AF.Tanh)
                nc.vector.tensor_mul(h_out, s[:, 4 * BC:6 * BC], tch[:])
                return c_new

            def recurrence(w0_sb, w1_sb, b1_sb, gx_d, n, outT, sts, psA, psB):
                """Wavefront over layers (L1 lags L0 by one step) and NCH
                independent batch chains interleaved so their serial
                elementwise chains overlap across engines.  h1 state lives
                directly in outT (bf16)."""
                nch = (n + CH - 1) // CH
                gx_tiles = {}
                outT4 = outT[:].rearrange("p (t k b) -> p t k b", k=KH, b=BL)
                b14 = b1_sb[:].rearrange("p (m b) -> p m b", b=BL)

                def issue_gx(c):
                    if c < nch:
                        gt = gxp.tile([128, CH, M8 * BL], BF16, tag="gx")
                        nc.sync.dma_start(gt[:], gx_d[:, c * CH:(c + 1) * CH, :])
                        gx_tiles[c] = gt

                for c in range(min(3, nch)):
                    issue_gx(c)

                for t in range(n + 1):
                    if t < n and t % CH == 0:
                        issue_gx(t // CH + 3)
                    for cn in range(NCH):
                        st = sts[cn]
                        b0 = cn * BC
                        p = f"q{cn}"
                        z1 = None
                        if t < n:
                            gxt = (gx_tiles[t // CH][:, t % CH, :]
                                   .rearrange("p (m b) -> p m b", b=BL)
                                   [:, :, b0:b0 + BC])
                            # L0: gx preload via identity, then weights
                            z0 = psA.tile([128, M8 * BC], F32, tag=p + "z0")
                            nc.tensor.matmul(z0[:], identb_sb[:], gxt,
                                             start=True, stop=False)
                            h0T = st["h0T"]
                            for m in range(M8):
                                for k in range(KH):
                                    j = m * KH + k
                                    nc.tensor.matmul(
                                        z0[:, m * BC:(m + 1) * BC],
                                        w0_sb[:, j * 128:(j + 1) * 128],
                                        h0T[:, k * BC:(k + 1) * BC],
                                        start=False,
                                        stop=(m == M8 - 1 and k == KH - 1),
                                    )
                            st["z0"] = z0
                        if t >= 1:
                            # L1 matmuls for step t-1; h1-reading MMs last
                            tp = t - 1
                            h0_in = st["h0_hist"][tp]
                            z1 = psB.tile([128, M8 * BC], F32, tag=p + "z1")
                            nc.tensor.matmul(z1[:], identb_sb[:],
                                             b14[:, :, b0:b0 + BC],
                                             start=True, stop=False)
                            for k in range(KHG):
                                if k < KH:
                                    rhs_all = h0_in[:, k * BC:(k + 1) * BC]
                                else:
                                    kk = k - KH
                                    rhs_all = (
                                        outT4[:, tp - 1, kk, b0:b0 + BC]
                                        if tp >= 1 else st["h1k"][kk])
                                for m in range(M8):
                                    j = m * KHG + k
                                    nc.tensor.matmul(
                                        z1[:, m * BC:(m + 1) * BC],
                                        w1_sb[:, j * 128:(j + 1) * 128],
                                        rhs_all,
                                        start=False,
                                        stop=(m == M8 - 1 and k == KHG - 1),
                                    )
                        if t < n:
                            h0n = statep.tile([128, 2 * BC], BF16, tag=p + "h0")
                            st["c0"] = gates(st.pop("z0"), st["c0"], p + "a",
                                             h0n[:])
                            st["h0_hist"][t] = h0n
                            if t - 2 in st["h0_hist"]:
                                del st["h0_hist"][t - 2]
                            st["h0T"] = h0n
                        if z1 is not None:
                            tp = t - 1
                            st["c1"] = gates(z1, st["c1"], p + "b",
                                             outT4[:, tp, :, b0:b0 + BC])
                for cn in range(NCH):
                    sts[cn]["h1k"] = [
                        outT4[:, n - 1, kk, cn * BC:(cn + 1) * BC]
                        for kk in range(KH)
                    ]
                return sts

            with (
                tc.tile_pool(name="psA", bufs=2, space="PSUM") as psA,
                tc.tile_pool(name="psB", bufs=2, space="PSUM") as psB,
            ):
                sts = [zeros_state(cn) for cn in range(NCH)]
                sts = recurrence(w0e_sb, w1e_sb, b1e_sb, gxe_d, ns, encT, sts,
                                 psA, psB)
                sts = recurrence(w0d_sb, w1d_sb, b1d_sb, gxd_d, nt, decT, sts,
                                 psA, psB)
                # timing-only extra repetitions of the steady-state loop;
                # chained through the live state so nothing is dead-code
                # eliminated (encT/decT rewritten identically afterwards).
                for _ in range(reps - 1):
                    sts = recurrence(w0e_sb, w1e_sb, b1e_sb, gxe_d, ns, encT,
                                     sts, psA, psB)
                    sts = recurrence(w0d_sb, w1d_sb, b1d_sb, gxd_d, nt, decT,
                                     sts, psA, psB)

            # ---------------- attention -------------------------------------
            nsc = ns // 128
            ntc = nt // 128
            with (
                tc.tile_pool(name="attn", bufs=2) as attp,
                tc.tile_pool(name="attnc", bufs=1) as attc,
                tc.tile_pool(name="attps", bufs=2, space="PSUM") as attps,
            ):
                enc_nat = attc.tile([128, BL, nsc, H], F32)
                encT4 = encT[:].rearrange("p (s k b) -> p s k b", k=KH, b=BL)
                decT4 = decT[:].rearrange("p (s k b) -> p s k b", k=KH, b=BL)
                for b in range(BL):
                    for k in range(KH):
                        for sc in range(nsc):
                            pt = attps.tile([128, 128], BF16, tag="ptb")
                            nc.tensor.transpose(
                                pt[:], encT4[:, sc * 128:(sc + 1) * 128, k, b],
                                identb_sb[:],
                            )
                            nc.scalar.copy(
                                enc_nat[:, b, sc, k * 128:(k + 1) * 128], pt[:]
                            )
                    for tcn in range(ntc):
                        ps_s = attps.tile([128, ns], F32, tag="ps_s")
                        for k in range(KH):
                            nc.tensor.matmul(
                                ps_s[:],
                                decT4[:, tcn * 128:(tcn + 1) * 128, k, b],
                                encT4[:, :, k, b],
                                start=(k == 0), stop=(k == KH - 1),
                            )
                        nmx = attp.tile([128, 1], F32, tag="nmx")
                        nc.vector.tensor_reduce(
                            nmx[:], ps_s[:], axis=AX.X, op=ALU.max, negate=True
                        )
                        wexp = attp.tile([128, ns], F32, tag="wexp")
                        den = attp.tile([128, 1], F32, tag="den")
                        nc.scalar.activation(
                            wexp[:], ps_s[:], AF.Exp, bias=nmx[:],
                            accum_out=den[:],
                        )
                        rden = attp.tile([128, 1], F32, tag="rden")
                        nc.vector.reciprocal(rden[:], den[:])
                        wn = attp.tile([128, ns], F32, tag="wn")
                        nc.vector.tensor_scalar_mul(wn[:], wexp[:], rden[:])
                        nc.sync.dma_start(
                            attnw[b, tcn * 128:(tcn + 1) * 128, :], wn[:]
                        )
                        wT = attp.tile([128, nsc * 128], F32, tag="wT")
                        for j in range(nsc):
                            ptw = attps.tile([128, 128], F32, tag="pt")
                            nc.tensor.transpose(
                                ptw[:], wn[:, j * 128:(j + 1) * 128], ident_sb[:]
                            )
                            nc.scalar.copy(wT[:, j * 128:(j + 1) * 128], ptw[:])
                        ps_v = attps.tile([128, H], F32, tag="ps_v")
                        for j in range(nsc):
                            nc.tensor.matmul(
                                ps_v[:],
                                wT[:, j * 128:(j + 1) * 128],
                                enc_nat[:, b, j, :],
                                start=(j == 0), stop=(j == nsc - 1),
                            )
                        vsb = attp.tile([128, H], F32, tag="vsb")
                        nc.scalar.copy(vsb[:], ps_v[:])
                        nc.sync.dma_start(
                            attnv[b, tcn * 128:(tcn + 1) * 128, :], vsb[:]
                        )
                nc.sync.dma_start(dect[:], decT[:])
    nc.compile()
    return nc


# ---------------------- host-side layout helpers ----------------------------

def _prep_xt(x):
    """(n, BL, 256) f32 -> (128, 2*n*BL) bf16, col = k*(n*BL) + t*BL + b."""
    n = x.shape[0]
    a = np.ascontiguousarray(x.transpose(2, 0, 1)).reshape(KH, 128, n * BL)
    return np.concatenate([a[0], a[1]], axis=1).astype(ml_dtypes.bfloat16)


def _prep_lhsT(Wp):
    """Permuted weight (1024, Kdim) -> (128, KT*8*128) bf16 lhsT tiles,
    col block j = m*KT + k."""
    Kd = Wp.shape[1]
    KT = Kd // 128
    t4 = np.ascontiguousarray(Wp.T).reshape(KT, 128, M8, 128)
    return np.ascontiguousarray(
        t4.transpose(1, 2, 0, 3)
    ).reshape(128, M8 * KT * 128).astype(ml_dtypes.bfloat16)


def _prep_shared(inputs, ns, nt):
    f = lambda k: np.asarray(inputs[k], np.float32)
    sh = {}
    # scale the g-gate rows (permuted rows 768:1024) by 2: tanh via sigmoid
    gsc = np.ones((G, 1), np.float32)
    gsc[3 * H:] = 2.0
    for tag, wih, whh, bih, bhh in (
        ("e", f("enc_Wih"), f("enc_Whh"), f("enc_bih"), f("enc_bhh")),
        ("d", f("dec_Wih"), f("dec_Whh"), f("dec_bih"), f("dec_bhh")),
    ):
        sh["w0" + tag] = _prep_lhsT(whh[0][GATE_ORDER] * gsc)
        sh["w1" + tag] = _prep_lhsT(
            np.concatenate([wih[1], whh[1]], axis=1)[GATE_ORDER] * gsc
        )
        sh["wi" + tag] = _prep_lhsT(wih[0][GATE_ORDER] * gsc)
        b0 = (bih[0] + bhh[0])[GATE_ORDER] * gsc[:, 0]
        sh["b0" + tag] = np.ascontiguousarray(b0.reshape(M8, 128).T)
        b1 = (bih[1] + bhh[1])[GATE_ORDER] * gsc[:, 0]
        b1r = b1.reshape(M8, 128).T          # (128, 8)
        sh["b1" + tag] = np.ascontiguousarray(
            np.repeat(b1r[:, :, None], BL, axis=2).reshape(128, M8 * BL)
        ).astype(ml_dtypes.bfloat16)
    sh["ident"] = np.eye(128, dtype=np.float32)
    sh["identb"] = np.eye(128, dtype=ml_dtypes.bfloat16)
    return sh


_BUILT = {}


def _get_nc(ns, nt):
    key = (ns, nt)
    if key not in _BUILT:
        _BUILT[key] = build_nc(ns, nt)
    return _BUILT[key]


def run(inputs, ns=S, nt=T):
    """Run the kernel; returns (responses, attn_w) full-shape."""
    nc = _get_nc(ns, nt)
    enc_in = np.asarray(inputs["enc_input"], np.float32)[:ns]
    dec_in = np.asarray(inputs["dec_input"], np.float32)[:nt]
    nb = enc_in.shape[1]
    ncores = nb // BL
    shared = _prep_shared(inputs, ns, nt)
    in_maps = []
    for c in range(ncores):
        m = dict(shared)
        sl = slice(c * BL, (c + 1) * BL)
        m["xte"] = _prep_xt(enc_in[:, sl, :])
        m["xtd"] = _prep_xt(dec_in[:, sl, :])
        in_maps.append(m)
    res = run_bass_kernel_spmd(nc, in_maps, list(range(ncores)))
    resp = np.empty((nt, nb, 2 * H), np.float32)
    attw = np.empty((nt, nb, ns), np.float32)
    for c in range(ncores):
        r = res.results[c]
        sl = slice(c * BL, (c + 1) * BL)
        dect = r["dect"].astype(np.float32).reshape(128, nt, KH, BL)
        resp[:, sl, 0:H] = np.ascontiguousarray(
            dect.transpose(1, 3, 2, 0)
        ).reshape(nt, BL, H)
        resp[:, sl, H:2 * H] = r["attnv"].transpose(1, 0, 2)
        attw[:, sl, :] = r["attnw"].transpose(1, 0, 2)
    return resp, attw


def kernel(**inputs):
    return run(inputs, S, T)
